# revision 26
# baseline (speedup 1.0000x reference)
"""Trainium2 Bass kernel for nn_Base_43765716746537 (gnn_message_passing).

Sharding: data-parallel over the turn axis T (16 turns / 8 cores = 2 per core).
Weights + schema-graph indices replicated; no collectives.

Per-core dataflow (everything in "feature-on-partition" transposed layout so
the contraction dim always sits on SBUF partitions):
  1. Build one-hot aggregation matrices A_col^T/A_tab^T [DB, U] on device from
     the (transposed) index/mask tensors with iota + is_equal accumulation,
     folding the 1/cnt masked-mean normalization in.
  2. masked mean == matmul: x^T[h,u] = feat[t]^T-contracted (feat as lhsT,
     A^T as moving operand), fp32r.
  3. Bidirectional 2-step GRU over [tab, col] as PSUM-accumulated matmuls
     (gi and gh concatenated into one accumulation group for the r/z gates)
     + DVE/ACT elementwise.
  4. attention over U via key-vector matmul + row softmax + DVE
     multiply-reduce; bilinear output head + log_softmax over U+K.
All matmuls run in float32r (fp22 reads, ~bf16 speed at N>=256, fp32 PSUM
accumulation).
"""

import os

import numpy as np

import concourse.bass as bass
import concourse.mybir as mybir
import concourse.tile as tile
from concourse import bacc
from concourse.bass_utils import run_bass_kernel_spmd

T, DB, H, U, MC, MT, L, K = 16, 512, 512, 1024, 8, 4, 64, 64
Hh = H // 2            # 256
NCORES = 8
TL = T // NCORES       # 2 turns per core
UK = U + K             # 1088
DT = DB // 128         # 4 db tiles
HT = H // 128          # 4 h tiles
GT = (3 * Hh) // 128   # 6 gate tiles

dt = mybir.dt
F32, F32R, I32, BF16 = dt.float32, dt.float32r, dt.int32, dt.bfloat16
OP = mybir.AluOpType
AF = mybir.ActivationFunctionType


def r(ap):
    return ap.bitcast(F32R)


def f(ap):
    return ap.bitcast(F32)


def flat_row(ap_handle, n):
    """View a DRAM tensor as a [1, n] row (row-major flatten)."""
    a = ap_handle.ap()
    return bass.AP(tensor=a.tensor, offset=a.offset, ap=[[0, 1], [1, n]])


def build_module():
    nc = bacc.Bacc()

    # ---------------- DRAM I/O (per core) ----------------
    feat_l = nc.dram_tensor("feat_l", [TL, DB, H], F32, kind="ExternalInput")
    key_l = nc.dram_tensor("key_l", [TL, H], F32, kind="ExternalInput")
    fft_l = nc.dram_tensor("fft_l", [TL, H, L], F32, kind="ExternalInput")
    wih_f_d = nc.dram_tensor("wih_f", [H, 3 * Hh], F32, kind="ExternalInput")
    wih_b_d = nc.dram_tensor("wih_b", [H, 3 * Hh], F32, kind="ExternalInput")
    whh_f_d = nc.dram_tensor("whh_f", [Hh, 3 * Hh], F32, kind="ExternalInput")
    whh_b_d = nc.dram_tensor("whh_b", [Hh, 3 * Hh], F32, kind="ExternalInput")
    bih_f_d = nc.dram_tensor("bih_f", [3 * Hh], F32, kind="ExternalInput")
    bhh_f_d = nc.dram_tensor("bhh_f", [3 * Hh], F32, kind="ExternalInput")
    bih_b_d = nc.dram_tensor("bih_b", [3 * Hh], F32, kind="ExternalInput")
    bhh_b_d = nc.dram_tensor("bhh_b", [3 * Hh], F32, kind="ExternalInput")
    wb_d = nc.dram_tensor("wb_t", [H, H], F32, kind="ExternalInput")
    bb_d = nc.dram_tensor("bb", [H], F32, kind="ExternalInput")
    kemb_d = nc.dram_tensor("kemb_t", [H, K], F32, kind="ExternalInput")
    cidx_d = nc.dram_tensor("cidx_t", [MC, U], I32, kind="ExternalInput")
    cmask_d = nc.dram_tensor("cmask_t", [MC, U], I32, kind="ExternalInput")
    tidx_d = nc.dram_tensor("tidx_t", [MT, U], I32, kind="ExternalInput")
    tmask_d = nc.dram_tensor("tmask_t", [MT, U], I32, kind="ExternalInput")
    amask_d = nc.dram_tensor("amask_l", [TL, U], I32, kind="ExternalInput")
    out_d = nc.dram_tensor("out_l", [TL, L, UK], F32, kind="ExternalOutput")

    with tile.TileContext(nc, pool_alloc_mode="queue") as tc:
        build_body(nc, tc, dict(
            feat_l=feat_l, key_l=key_l, fft_l=fft_l,
            wih_f=wih_f_d, wih_b=wih_b_d, whh_f=whh_f_d, whh_b=whh_b_d,
            bih_f=bih_f_d, bhh_f=bhh_f_d, bih_b=bih_b_d, bhh_b=bhh_b_d,
            wb=wb_d, bb=bb_d, kemb=kemb_d,
            cidx=cidx_d, cmask=cmask_d, tidx=tidx_d, tmask=tmask_d,
            amask=amask_d, out=out_d,
        ))
    nc.compile()
    return nc


def build_body(nc, tc, d):
    stage = int(os.environ.get("KSTAGE", "99"))

    def dump(tile_ap, rows, cols):
        o = d["out"].ap()
        dst = bass.AP(tensor=o.tensor, offset=o.offset, ap=[[cols, rows], [1, cols]])
        nc.gpsimd.dma_start(out=dst, in_=tile_ap.bitcast(F32))

    consts = tc.alloc_tile_pool(name="consts", bufs=1)

    # ---------------- stage 0: load replicated weights / biases ----------------
    wf = [consts.tile([128, 3 * Hh], F32R, tag=f"wf{k}", name=f"wf{k}") for k in range(HT)]
    wbk = [consts.tile([128, 3 * Hh], F32R, tag=f"wbk{k}", name=f"wbk{k}") for k in range(HT)]
    for k in range(HT):
        nc.gpsimd.dma_start(out=wf[k], in_=r(d["wih_f"][k * 128:(k + 1) * 128, :]))
        nc.gpsimd.dma_start(out=wbk[k], in_=r(d["wih_b"][k * 128:(k + 1) * 128, :]))
    uf = [consts.tile([128, 3 * Hh], F32R, tag=f"uf{k}", name=f"uf{k}") for k in range(2)]
    ub = [consts.tile([128, 3 * Hh], F32R, tag=f"ub{k}", name=f"ub{k}") for k in range(2)]
    for k in range(2):
        nc.gpsimd.dma_start(out=uf[k], in_=r(d["whh_f"][k * 128:(k + 1) * 128, :]))
        nc.gpsimd.dma_start(out=ub[k], in_=r(d["whh_b"][k * 128:(k + 1) * 128, :]))
    wbt = [consts.tile([128, H], F32R, tag=f"wbt{k}", name=f"wbt{k}") for k in range(HT)]
    kem = [consts.tile([128, K], F32R, tag=f"kem{k}", name=f"kem{k}") for k in range(HT)]
    for k in range(HT):
        nc.gpsimd.dma_start(out=wbt[k], in_=r(d["wb"][k * 128:(k + 1) * 128, :]))
        nc.gpsimd.dma_start(out=kem[k], in_=r(d["kemb"][k * 128:(k + 1) * 128, :]))

    # biases as [128, 6] (column g-tile), plus bb as [128, 4]
    def load_bias(name):
        t_ = consts.tile([128, GT], F32, tag=name, name=name)
        nc.gpsimd.dma_start(out=t_, in_=d[name].ap().rearrange("(g p) -> p g", p=128))
        return t_
    bih_f = load_bias("bih_f"); bhh_f = load_bias("bhh_f")
    bih_b = load_bias("bih_b"); bhh_b = load_bias("bhh_b")
    bcomb_f = consts.tile([128, GT], F32, tag="bcomb_f", name="bcomb_f")
    bcomb_b = consts.tile([128, GT], F32, tag="bcomb_b", name="bcomb_b")
    nc.vector.tensor_add(bcomb_f, bih_f, bhh_f)
    nc.vector.tensor_add(bcomb_b, bih_b, bhh_b)
    bb4 = consts.tile([128, HT], F32, tag="bb4", name="bb4")
    nc.gpsimd.dma_start(out=bb4, in_=d["bb"].ap().rearrange("(g p) -> p g", p=128))

    # key^T [128, TL, HT], pre-scaled by 0.5 (db_emb is stored unscaled = o0+o1)
    keyt = consts.tile([128, TL, HT], F32R, tag="keyt", name="keyt")
    nc.gpsimd.dma_start(out=keyt, in_=r(d["key_l"].ap().rearrange("t (k p) -> p t k", p=128)))
    nc.vector.tensor_scalar_mul(keyt, f(keyt), 0.5)

    # final_feature^T tiles [128, L] per (t, k)
    fft = [[consts.tile([128, L], F32, tag=f"fft{t}{k}", name=f"fft{t}{k}")
            for k in range(HT)] for t in range(TL)]
    for t in range(TL):
        for k in range(HT):
            nc.gpsimd.dma_start(out=fft[t][k], in_=d["fft_l"][t, k * 128:(k + 1) * 128, :])

    # attention mask as multiplicative 0/1 row (bf16 is exact for 0/1)
    mask01 = consts.tile([1, TL * U], BF16, tag="mask01", name="mask01")

    # iota column values p + 128*j
    iof = consts.tile([128, DT], F32, tag="iof", name="iof")

    # A^T accumulators (live through the gather stage)
    atp = tc.alloc_tile_pool(name="atp", bufs=1)
    acol = [atp.tile([128, U], F32R, tag=f"acol{p}", name=f"acol{p}") for p in range(DT)]
    atab = [atp.tile([128, U], F32R, tag=f"atab{p}", name=f"atab{p}") for p in range(DT)]

    psp = tc.alloc_tile_pool(name="psp", bufs=8, space="PSUM")

    # ---------------- stage A: build A^T (scoped pool, released) ----------------
    ab = tc.alloc_tile_pool(name="abuild", bufs=1)

    ioi = ab.tile([128, DT], I32, tag="ioi", name="ioi")
    nc.gpsimd.iota(ioi, pattern=[[128, DT]], base=0, channel_multiplier=1)
    nc.vector.tensor_copy(iof, ioi)

    for t_ in range(TL):
        amask_i = ab.tile([1, U], I32, tag="amask_i", name=f"amask_i{t_}")
        a_ = d["amask"].ap()
        nc.gpsimd.dma_start(out=amask_i, in_=bass.AP(tensor=a_.tensor, offset=a_.offset + t_ * U,
                                                   ap=[[0, 1], [1, U]]))
        nc.vector.tensor_copy(mask01[:, t_ * U:(t_ + 1) * U], amask_i)

    ones_f = ab.tile([MC, 1], F32, tag="ones_f", name="ones_f")
    nc.vector.memset(ones_f, 1.0)
    ones8 = ab.tile([MC, 1], F32R, tag="ones8", name="ones8")
    nc.vector.tensor_copy(ones8, ones_f)

    def build_AT(idx_d, mask_d, M, acc, nm):
        # load transposed idx/mask rows (tags shared between col/tab passes)
        idx_i = ab.tile([M, U], I32, tag="idx_i", name=f"{nm}idx_i")
        msk_i = ab.tile([M, U], I32, tag="msk_i", name=f"{nm}msk_i")
        nc.gpsimd.dma_start(out=idx_i, in_=idx_d.ap())
        nc.gpsimd.dma_start(out=msk_i, in_=mask_d.ap())
        # idx converts in place as plain f32 (never consumed by a matmul);
        # the mask gets a plain f32 copy for arithmetic plus an f32r-typed
        # copy for the cnt matmul (verifier tracks producers per location)
        idx_f = idx_i.bitcast(F32)
        nc.vector.tensor_copy(idx_f, idx_i)
        msk_g = msk_i.bitcast(F32)
        nc.vector.tensor_copy(msk_g, msk_i)
        msk_f = ab.tile([M, U], F32R, tag="msk_r", name=f"{nm}msk_r")
        nc.vector.tensor_copy(msk_f, msk_g)
        # masked index: idx' = (idx + 1) * mask - 1  (-1 never matches iota)
        nc.vector.tensor_scalar_add(idx_f, idx_f, 1.0)
        nc.vector.tensor_mul(idx_f, idx_f, msk_g)
        nc.vector.tensor_scalar_sub(idx_f, idx_f, 1.0)
        # cnt = max(sum_m mask, 1);  rcnt broadcast over partitions
        pc = [psp.tile([1, 512], F32, tag="ps", name=f"{nm}pc{h}") for h in range(2)]
        for h in range(2):
            nc.tensor.matmul(pc[h], r(ones8[0:M, :]), r(msk_f[:, h * 512:(h + 1) * 512]),
                             start=True, stop=True)
        cnt = ab.tile([1, U], F32, tag="cnt", name=f"{nm}cnt")
        for h in range(2):
            nc.vector.tensor_scalar_max(cnt[:, h * 512:(h + 1) * 512], pc[h], 1.0)
        rcnt = ab.tile([1, U], F32, tag="rcnt", name=f"{nm}rcnt")
        nc.vector.reciprocal(rcnt, cnt)
        rcb = ab.tile([128, U], F32, tag="rcb", name=f"{nm}rcb")
        nc.gpsimd.partition_broadcast(rcb, rcnt)
        # accumulate one-hots
        for m in range(M):
            stage = ab.tile([1, U], F32, tag="stage", name=f"{nm}stage{m}")
            nc.gpsimd.dma_start(out=stage, in_=idx_f[m:m + 1, :])
            ib = ab.tile([128, U], F32, tag=f"ib{m % 2}", name=f"{nm}ib{m}")
            nc.gpsimd.partition_broadcast(ib, stage)
            for p in range(DT):
                if m == 0:
                    nc.vector.tensor_scalar(acc[p], ib, iof[:, p:p + 1], None, OP.is_equal)
                else:
                    nc.vector.scalar_tensor_tensor(acc[p], ib, iof[:, p:p + 1], f(acc[p]),
                                                   OP.is_equal, OP.add)
        # fold 1/cnt into A^T
        for p in range(DT):
            nc.vector.tensor_mul(acc[p], f(acc[p]), rcb)

    if stage >= 1:
        build_AT(d["cidx"], d["cmask"], MC, acol, "c")
        build_AT(d["tidx"], d["tmask"], MT, atab, "t")
    ab.release()
    if stage == 0:
        dump(wf[0], 128, 768)
    if stage == 1:
        dump(acol[0], 128, U)

    # ---------------- stages B+C interleaved per turn ----------------
    xp = tc.alloc_tile_pool(name="xp", bufs=1)
    fp = tc.alloc_tile_pool(name="featp", bufs=1)
    gp = tc.alloc_tile_pool(name="grup", bufs=1)
    tp = tc.alloc_tile_pool(name="tmpp", bufs=1)
    mp = tc.alloc_tile_pool(name="miscp", bufs=1)
    sp = tc.alloc_tile_pool(name="scrp", bufs=2)

    demb = [[None] * HT for _ in range(TL)]
    xcol = [[None] * HT for _ in range(TL)]
    xtab = [[None] * HT for _ in range(TL)]

    for t in range(TL if stage >= 2 else 0):
        # ---- gather (masked mean as matmul) ----
        fb = [fp.tile([128, H], F32R, tag=f"fb{dd}", name=f"fb{t}{dd}") for dd in range(DT)]
        for dd in range(DT):
            nc.gpsimd.dma_start(out=fb[dd], in_=r(d["feat_l"][t, dd * 128:(dd + 1) * 128, :]))
        for ht in range(HT):
            xcol[t][ht] = xp.tile([128, U], F32R, tag=f"xc{ht}", name=f"xc{t}{ht}")
            xtab[t][ht] = xp.tile([128, U], F32R, tag=f"xt{ht}", name=f"xt{t}{ht}")
        for ht in range(HT):
            pg = {}
            for mat in range(2):
                for hf in range(2):
                    pg[(mat, hf)] = psp.tile([128, 512], F32, tag="ps",
                                             name=f"pg{t}{ht}{mat}{hf}")
            for dd in range(DT):
                lhs = r(fb[dd][:, ht * 128:(ht + 1) * 128])
                for mat, acc in ((0, acol), (1, atab)):
                    for hf in range(2):
                        nc.tensor.matmul(pg[(mat, hf)], lhs,
                                         r(acc[dd][:, hf * 512:(hf + 1) * 512]),
                                         start=(dd == 0), stop=(dd == DT - 1))
            for hf in range(2):
                nc.vector.tensor_copy(xcol[t][ht][:, hf * 512:(hf + 1) * 512], pg[(0, hf)])
                nc.vector.tensor_copy(xtab[t][ht][:, hf * 512:(hf + 1) * 512], pg[(1, hf)])

        if stage == 2:
            if t == 0:
                dump(xcol[0][0], 128, U)
            continue

        for ht in range(HT):
            demb[t][ht] = gp.tile([128, U], F32R, tag=f"de{ht}", name=f"de{t}{ht}")

        for hf in range(2):
            sl = slice(hf * 512, (hf + 1) * 512)

            def gi_mm(x, w, gts, extra=None):
                ps = []
                for gt in gts:
                    pt = psp.tile([128, 512], F32, tag="ps", name=f"pgi{t}{hf}{gt}")
                    nmm = HT + (2 if extra is not None else 0)
                    i = 0
                    for k in range(HT):
                        nc.tensor.matmul(pt, r(w[k][:, gt * 128:(gt + 1) * 128]),
                                         r(x[k]), start=(i == 0), stop=(i == nmm - 1))
                        i += 1
                    if extra is not None:
                        u_, hsrc = extra
                        for k in range(2):
                            nc.tensor.matmul(pt, r(u_[k][:, gt * 128:(gt + 1) * 128]),
                                             r(hsrc[k]), start=(i == 0),
                                             stop=(i == nmm - 1))
                            i += 1
                    ps.append(pt)
                return ps

            def cell1(x, bcomb, bih, bhh, w, hname):
                """first GRU step (h0 = 0): h1 = (1 - z) * n"""
                pg = gi_mm(x, w, range(GT))
                h1, rg, zg = [], [], []
                for j in range(2):
                    rj = tp.tile([128, 512], F32, tag=f"r{j}", name=f"r_{hname}{t}{hf}{j}")
                    nc.scalar.activation(rj, pg[j], AF.Sigmoid, bias=bcomb[:, j:j + 1])
                    rg.append(rj)
                for j in range(2):
                    zj = tp.tile([128, 512], F32, tag=f"z{j}", name=f"z_{hname}{t}{hf}{j}")
                    nc.scalar.activation(zj, pg[2 + j], AF.Sigmoid, bias=bcomb[:, 2 + j:3 + j])
                    zg.append(zj)
                for j in range(2):
                    sj = tp.tile([128, 512], F32, tag=f"s{j}", name=f"s_{hname}{t}{hf}{j}")
                    nc.vector.scalar_tensor_tensor(sj, rg[j], bhh[:, 4 + j:5 + j], pg[4 + j],
                                                   OP.mult, OP.add)
                    nj = tp.tile([128, 512], F32, tag=f"n{j}", name=f"n_{hname}{t}{hf}{j}")
                    nc.scalar.activation(nj, sj, AF.Tanh, bias=bih[:, 4 + j:5 + j])
                    hj = gp.tile([128, 512], F32R, tag=f"h1{hname}{j}", name=f"h_{hname}{t}{hf}{j}")
                    ej = tp.tile([128, 512], F32, tag=f"w{j}", name=f"e_{hname}{t}{hf}{j}")
                    nc.vector.tensor_mul(ej, zg[j], nj)
                    nc.vector.tensor_sub(hj, nj, ej)
                    h1.append(hj)
                return h1

            def cell2(x, hprev, bcomb, bih, bhh, w, u_, hname, dtiles):
                """second step; writes (hprev + h2) straight into demb tiles"""
                prz = gi_mm(x, w, range(4), extra=(u_, hprev))
                pin = gi_mm(x, w, (4, 5))
                phn = []
                for j in range(2):
                    pt = psp.tile([128, 512], F32, tag="ps", name=f"phn{t}{hf}{hname}{j}")
                    for k in range(2):
                        nc.tensor.matmul(pt, r(u_[k][:, (4 + j) * 128:(5 + j) * 128]),
                                         r(hprev[k]), start=(k == 0), stop=(k == 1))
                    phn.append(pt)
                rg, zg = [], []
                for j in range(2):
                    rj = tp.tile([128, 512], F32, tag=f"r{j}", name=f"r2_{hname}{t}{hf}{j}")
                    nc.scalar.activation(rj, prz[j], AF.Sigmoid, bias=bcomb[:, j:j + 1])
                    rg.append(rj)
                for j in range(2):
                    zj = tp.tile([128, 512], F32, tag=f"z{j}", name=f"z2_{hname}{t}{hf}{j}")
                    nc.scalar.activation(zj, prz[2 + j], AF.Sigmoid, bias=bcomb[:, 2 + j:3 + j])
                    zg.append(zj)
                for j in range(2):
                    s1 = tp.tile([128, 512], F32, tag=f"s{j}", name=f"s2_{hname}{t}{hf}{j}")
                    nc.vector.scalar_tensor_tensor(s1, phn[j], bhh[:, 4 + j:5 + j], rg[j],
                                                   OP.add, OP.mult)
                    s2 = tp.tile([128, 512], F32, tag=f"w{j}", name=f"s3_{hname}{t}{hf}{j}")
                    nc.vector.tensor_add(s2, s1, pin[j])
                    nj = tp.tile([128, 512], F32, tag=f"n{j}", name=f"n2_{hname}{t}{hf}{j}")
                    nc.scalar.activation(nj, s2, AF.Tanh, bias=bih[:, 4 + j:5 + j])
                    # demb = hprev + h2 = (hprev + n) + z*(hprev - n)
                    dj = tp.tile([128, 512], F32, tag=f"w{j}", name=f"d_{hname}{t}{hf}{j}")
                    nc.vector.tensor_sub(dj, f(hprev[j]), nj)
                    zj2 = tp.tile([128, 512], F32, tag=f"r{j}", name=f"zd_{hname}{t}{hf}{j}")
                    nc.vector.tensor_mul(zj2, zg[j], dj)
                    pn = tp.tile([128, 512], F32, tag=f"s{j}", name=f"pn_{hname}{t}{hf}{j}")
                    nc.vector.tensor_add(pn, f(hprev[j]), nj)
                    nc.vector.tensor_add(dtiles[j][:, sl], pn, zj2)

            x1 = [xtab[t][k][:, sl] for k in range(HT)]
            x2 = [xcol[t][k][:, sl] for k in range(HT)]

            f1 = cell1(x1, bcomb_f, bih_f, bhh_f, wf, "f")
            b1 = cell1(x2, bcomb_b, bih_b, bhh_b, wbk, "b")
            cell2(x2, f1, bcomb_f, bih_f, bhh_f, wf, uf, "f", demb[t][0:2])
            cell2(x1, b1, bcomb_b, bih_b, bhh_b, wbk, ub, "b", demb[t][2:4])

        if stage == 3:
            if t == 0:
                dump(demb[0][0], 128, U)
            continue

        # ---------- attention ----------
        pl = [psp.tile([1, 512], F32, tag="ps", name=f"pl{t}{h}") for h in range(2)]
        for h in range(2):
            for k in range(HT):
                nc.tensor.matmul(pl[h], r(keyt[:, t, k:k + 1]),
                                 r(demb[t][k][:, h * 512:(h + 1) * 512]),
                                 start=(k == 0), stop=(k == HT - 1))
        logits = sp.tile([1, U], F32, tag="esc", name=f"logits{t}", bufs=3)
        for h in range(2):
            nc.vector.tensor_copy(logits[:, h * 512:(h + 1) * 512], pl[h])
        if stage == 4:
            if t == 0:
                dump(logits, 1, U)
            continue
        mx = mp.tile([1, 1], F32, tag="mx", name=f"mx{t}")
        nc.vector.reduce_max(mx, logits, axis=mybir.AxisListType.X)
        nmx = mp.tile([1, 1], F32, tag="nmx", name=f"nmx{t}")
        nc.vector.tensor_scalar_mul(nmx, mx, -1.0)
        wexp = mp.tile([1, U], F32, tag="wexp", name=f"wexp{t}")
        nc.scalar.activation(wexp, logits, AF.Exp, bias=nmx)
        # zero out masked entries, then normalize
        nc.vector.tensor_mul(wexp, wexp, mask01[:, t * U:(t + 1) * U])
        sume = mp.tile([1, 1], F32, tag="sume", name=f"sume{t}")
        nc.vector.reduce_sum(sume, wexp, axis=mybir.AxisListType.X)
        rsum = mp.tile([1, 1], F32, tag="rsum", name=f"rsum{t}")
        nc.vector.reciprocal(rsum, sume)
        nc.vector.tensor_scalar_mul(wexp, wexp, rsum)
        wb_ = mp.tile([128, U], F32, tag="wb_", name=f"wb_{t}")
        nc.gpsimd.partition_broadcast(wb_, wexp)
        if stage == 5:
            if t == 0:
                dump(wb_, 128, U)
            continue
        attn = mp.tile([128, HT], F32, tag="attn", name=f"attn{t}")
        for k in range(HT):
            scr = sp.tile([128, U], F32, tag="scr", name=f"scr{t}{k}", bufs=1)
            nc.vector.tensor_mul(scr, f(demb[t][k]), wb_)
            nc.vector.reduce_sum(attn[:, k:k + 1], scr, axis=mybir.AxisListType.X)
        nc.vector.tensor_scalar_mul(attn, attn, 0.5)
        if stage == 6:
            if t == 0:
                dump(attn, 128, HT)
            continue

        # ---------- output head ----------
        feat_r = mp.tile([128, HT, L], F32R, tag="feat_r", name=f"feat_r{t}")
        for k in range(HT):
            nc.vector.tensor_scalar_add(feat_r[:, k, :], fft[t][k], attn[:, k:k + 1])
        if stage == 7:
            if t == 0:
                dump(feat_r, 128, HT * L)
            continue
        fft_o = mp.tile([128, HT, L], F32R, tag="fft_o", name=f"fft_o{t}")
        for kt in range(HT):
            pf = psp.tile([128, L], F32, tag="ps", name=f"pf{t}{kt}")
            for k in range(HT):
                nc.tensor.matmul(pf, r(wbt[k][:, kt * 128:(kt + 1) * 128]),
                                 r(feat_r[:, k, :]), start=(k == 0), stop=(k == HT - 1))
            nc.scalar.activation(fft_o[:, kt, :], pf, AF.Tanh, bias=bb4[:, kt:kt + 1])
        if stage == 8:
            if t == 0:
                dump(fft_o, 128, HT * L)
            continue

        prob = mp.tile([L, UK], F32, tag="prob", name=f"prob{t}")
        for h in range(2):
            pd = psp.tile([L, 512], F32, tag="ps", name=f"pd{t}{h}")
            for k in range(HT):
                nc.tensor.matmul(pd, r(fft_o[:, k, :]),
                                 r(demb[t][k][:, h * 512:(h + 1) * 512]),
                                 start=(k == 0), stop=(k == HT - 1))
            nc.vector.tensor_scalar_mul(prob[:, h * 512:(h + 1) * 512], pd, 0.5)
        pk = psp.tile([L, K], F32, tag="ps", name=f"pk{t}")
        for k in range(HT):
            nc.tensor.matmul(pk, r(fft_o[:, k, :]), r(kem[k]),
                             start=(k == 0), stop=(k == HT - 1))
        nc.vector.tensor_copy(prob[:, U:UK], pk)
        if stage == 9:
            if t == 0:
                dump(prob, L, UK)
            continue

        # ---------- log_softmax over U+K ----------
        mx2 = mp.tile([L, 1], F32, tag="mx2", name=f"mx2{t}")
        nc.vector.reduce_max(mx2, prob, axis=mybir.AxisListType.X)
        nmx2 = mp.tile([L, 1], F32, tag="nmx2", name=f"nmx2{t}")
        nc.vector.tensor_scalar_mul(nmx2, mx2, -1.0)
        esc = sp.tile([L, UK], F32, tag="esc", name=f"esc{t}", bufs=3)
        se = mp.tile([L, 1], F32, tag="se", name=f"se{t}")
        nc.scalar.activation(esc, prob, AF.Exp, bias=nmx2, accum_out=se)
        lse = mp.tile([L, 1], F32, tag="lse", name=f"lse{t}")
        nc.scalar.activation(lse, se, AF.Ln)
        # hw quirk: tensor_scalar subtract with an AP scalar silently
        # drops the scalar, so compute the negated shift and add it
        shift = mp.tile([L, 1], F32, tag="shift", name=f"shift{t}")
        nc.vector.tensor_add(shift, mx2, lse)
        nc.vector.tensor_scalar_mul(shift, shift, -1.0)
        if stage == 10 and t == 0:
            dump(shift, L, 1)
            continue
        if stage == 11 and t == 0:
            dump(se, L, 1)
            continue
        if stage == 12 and t == 0:
            dump(mx2, L, 1)
            continue
        outsb = sp.tile([L, UK], F32, tag="esc", name=f"outsb{t}", bufs=3)
        nc.vector.tensor_scalar(outsb, prob, shift, None, OP.add)
        nc.gpsimd.dma_start(out=d["out"][t], in_=outsb)

    for p in (sp, mp, tp, gp, fp, xp, psp, atp, consts):
        p.release()


_NC_CACHE = []


def get_module():
    if not _NC_CACHE:
        _NC_CACHE.append(build_module())
    return _NC_CACHE[0]


def make_in_maps(inputs):
    """Host-side layout prep + T-sharding. Pure transposes/slices."""
    feat = np.ascontiguousarray(inputs["feat"], np.float32)
    key = np.ascontiguousarray(inputs["key"], np.float32)
    fft_full = np.ascontiguousarray(np.transpose(inputs["final_feature"], (0, 2, 1)), np.float32)
    rep = dict(
        wih_f=np.ascontiguousarray(inputs["Wih_f"].T, np.float32),
        wih_b=np.ascontiguousarray(inputs["Wih_b"].T, np.float32),
        whh_f=np.ascontiguousarray(inputs["Whh_f"].T, np.float32),
        whh_b=np.ascontiguousarray(inputs["Whh_b"].T, np.float32),
        bih_f=np.ascontiguousarray(inputs["bih_f"], np.float32),
        bhh_f=np.ascontiguousarray(inputs["bhh_f"], np.float32),
        bih_b=np.ascontiguousarray(inputs["bih_b"], np.float32),
        bhh_b=np.ascontiguousarray(inputs["bhh_b"], np.float32),
        wb_t=np.ascontiguousarray(inputs["Wb"].T, np.float32),
        bb=np.ascontiguousarray(inputs["bb"], np.float32),
        kemb_t=np.ascontiguousarray(inputs["Kemb"].T, np.float32),
        cidx_t=np.ascontiguousarray(inputs["col_idx"].T, np.int32),
        cmask_t=np.ascontiguousarray(inputs["col_mask"].T, np.int32),
        tidx_t=np.ascontiguousarray(inputs["tab_idx"].T, np.int32),
        tmask_t=np.ascontiguousarray(inputs["tab_mask"].T, np.int32),
    )
    in_maps = []
    for c in range(NCORES):
        s = slice(c * TL, (c + 1) * TL)
        m = dict(rep)
        m["feat_l"] = feat[s]
        m["key_l"] = key[s]
        m["fft_l"] = fft_full[s]
        m["amask_l"] = np.ascontiguousarray(inputs["attn_mask"][s], np.int32)
        in_maps.append(m)
    return in_maps


def kernel(**inputs):
    nc = get_module()
    in_maps = make_in_maps(inputs)
    res = run_bass_kernel_spmd(nc, in_maps, list(range(NCORES)))
    return np.concatenate([res.results[c]["out_l"] for c in range(NCORES)], axis=0)
